# revision 1
# baseline (speedup 1.0000x reference)
"""Multi-head attention (B=2, S=2048, D=1024, H=16) on 8 TRN2 NeuronCores.

Sharding: core c -> (batch b = c//4, head-group g = c%4). Each core computes
the attention output restricted to its batch and its 4 heads (a 256-wide
slice of the model dim), including the row-parallel output projection
partial product. Host sums the 4 partials per batch and adds bo.

Device-side layouts (everything transposed so no on-device transposes are
needed):
  xq/xk/xv  bf16 [1025, 2048]  = x[b].T with a trailing ones row (bias trick)
  wq/wk/wv  bf16 [1025, 256]   = W[g-slice, :].T with trailing bias row
  wo        bf16 [256, 1024]   = Wo[:, g-slice].T
  outT      f32  [1024, 2048]  = (Wo_g @ ctxn_g^T) partial, host transposes

Pipeline per core:
  Q^T,K^T = W x^T            (PE, contraction over model dim, psum accum)
  V       = x^T-stationary   (natural [s, d] layout, +ones column -> Z sums)
  per head: scores^T[k,q] = K_h^T-stationary @ Q_h^T   (psum [128,2048])
            attn = exp(scores/8)                        (ACT, psum->sbuf bf16)
            ctx_aug^T[d+1,q] += V_aug^T-stationary @ attn (psum accum)
            ctxn^T = ctx^T * recip(Z) broadcast          (DVE + gpsimd bcast)
  outT[oc] = wo-stationary @ ctxn^T                      (PE, psum accum)
"""

import numpy as np
import ml_dtypes

from concourse import bacc, tile, mybir
from concourse.bass_utils import run_bass_kernel_spmd

BF16 = mybir.dt.bfloat16
F32 = mybir.dt.float32

S = 2048      # sequence length
D = 1024      # model dim
DG = 256      # per-core head-group width (4 heads x 64)
DK = 64       # head dim
NH = 4        # heads per core
MT = 8        # model-dim contraction tiles (1024 / 128)
QC = 4        # q chunks of 512
KC = 16       # k chunks of 128
N_CORES = 8


def _copy_evict(nc, idx, out_ap, in_ap):
    """Alternate PSUM->SBUF evictions between DVE and ACT to split the load."""
    if idx % 2 == 0:
        nc.vector.tensor_copy(out_ap, in_ap)
    else:
        nc.scalar.copy(out_ap, in_ap)


def _emit(nc, pools, dram):
    persist, xp, wp, wop, attnp, zp, outp, ps, ctxps, smallps = pools
    xq, xk, xv, wq, wk, wv, bT, wo, outT0, outT1 = dram
    HS = S // 2  # 1024-wide half grains

    # persistent tiles for this iteration
    qt = [persist.tile([128, S], BF16, tag=f"qt{i}", name=f"qt{i}") for i in range(2)]
    kt = [persist.tile([128, S], BF16, tag=f"kt{i}", name=f"kt{i}") for i in range(2)]
    ctxn = [persist.tile([128, S], BF16, tag=f"ctxn{i}", name=f"ctxn{i}") for i in range(2)]
    vaug = persist.tile([128, KC, NH, DK + 1], BF16, tag="vaug", name="vaug")
    ones = persist.tile([1, S], BF16, tag="ones", name="ones")

    nc.vector.memset(ones[:], 1.0)
    bt = persist.tile([1, 3 * DG], BF16, tag="bt", name="bt")
    # ones columns of V_aug (softmax denominator accumulates here)
    nc.vector.memset(vaug[:, :, :, DK:DK + 1], 1.0)

    # ---------------- Q^T / K^T projections ----------------
    ev = 0
    # DMA order: low-column halves of xq AND xk first -> first scores matmuls
    # (which need only qt/kt hf0 grains) can start ~14us earlier.
    wts, xts = {}, {}
    for key, wdr, xdr in (("q", wq, xq), ("k", wk, xk)):
        wt, xt = [], []
        for m in range(MT):
            t = wp.tile([128, DG], BF16, tag="w", name="w")
            nc.sync.dma_start(t[:], wdr[m * 128:(m + 1) * 128, :])
            wt.append(t)
            xt.append(xp.tile([128, S], BF16, tag="x", name="x"))
        for m in range(MT):
            nc.sync.dma_start(xt[m][:, 0:HS], xdr[m * 128:(m + 1) * 128, 0:HS])
        wts[key], xts[key] = wt, xt
    nc.sync.dma_start(bt[:], bT[:])
    for key, xdr in (("q", xq), ("k", xk)):
        for m in range(MT):
            nc.sync.dma_start(xts[key][m][:, HS:S],
                              xdr[m * 128:(m + 1) * 128, HS:S])

    for bofs, (key, outsb) in enumerate((("q", qt), ("k", kt))):
        wt, xt = wts[key], xts[key]
        for dch in range(1):
            for hf in range(2):
                psum = ps.tile([128, HS], F32, tag="ps", name="ps")
                for m in range(MT):
                    for qc in range(2):
                        nc.tensor.matmul(
                            psum[:, qc * 512:(qc + 1) * 512],
                            wt[m][:, dch * 128:(dch + 1) * 128],
                            xt[m][:, hf * HS + qc * 512:hf * HS + (qc + 1) * 512],
                            start=(m == 0), stop=False)
                for qc in range(2):
                    nc.tensor.matmul(
                        psum[:, qc * 512:(qc + 1) * 512],
                        bt[:, bofs * DG + dch * 128:bofs * DG + (dch + 1) * 128],
                        ones[:, hf * HS + qc * 512:hf * HS + (qc + 1) * 512],
                        start=False, stop=True)
                nc.vector.tensor_copy(outsb[dch][:, hf * HS:(hf + 1) * HS],
                                      psum[:])

    # ---------------- V projection (natural [s, d] layout) ----------------
    wvt, xvt = [], []
    for m in range(MT):
        t = wp.tile([128, DG], BF16, tag="w", name="w")
        nc.sync.dma_start(t[:], wv[m * 128:(m + 1) * 128, :])
        wvt.append(t)
        xvt.append(xp.tile([128, S], BF16, tag="x", name="x"))
    for hf in range(2):
        for m in range(MT):
            nc.sync.dma_start(
                xvt[m][:, hf * HS:(hf + 1) * HS],
                xv[m * 128:(m + 1) * 128, hf * HS:(hf + 1) * HS])
    def vproj_grain(sc):
        vps = smallps.tile([128, NH, DK], F32, tag="sm", name="vps")
        for m in range(MT):
            nc.tensor.matmul(
                vps[:, :, :],
                xvt[m][:, sc * 128:(sc + 1) * 128],
                wvt[m][:],
                start=(m == 0), stop=False)
        nc.tensor.matmul(
            vps[:, :, :],
            ones[:, sc * 128:(sc + 1) * 128],
            bt[:, 2 * DG:3 * DG],
            start=False, stop=True)
        nc.vector.tensor_copy(vaug[:, sc, :, 0:DK], vps[:, :, :])

    # out-projection weights (DMA sits behind the x tiles; needed much later)
    wot = []
    for dch in range(2):
        t = wop.tile([128, D], BF16, tag="wo", name="wo")
        nc.sync.dma_start(t[:], wo[dch * 128:(dch + 1) * 128, :])
        wot.append(t)

    # ---------------- attention per head ----------------
    def outproj_grain(dch, oc, qp, outT, ev):
        """A [128, 1024] out-projection pair: two 512-wide psum grains,
        evicted into one SBUF tile, shipped with a single DMA."""
        osb = outp.tile([128, HS], BF16, tag="out", name="out")
        for j in range(2):
            q4 = 2 * qp + j
            ops = smallps.tile([128, 512], F32, tag="sm", name="ops")
            nc.tensor.matmul(
                ops[:], wot[dch][:, oc * 128:(oc + 1) * 128],
                ctxn[dch][:, q4 * 512:(q4 + 1) * 512],
                start=True, stop=True)
            nc.vector.tensor_copy(osb[:, j * 512:(j + 1) * 512], ops[:])
        nc.sync.dma_start(
            outT[oc * 128:(oc + 1) * 128, qp * HS:(qp + 1) * HS], osb[:])

    # out-grains of finished ctxn regions are interleaved into later heads
    from collections import deque
    pending = deque()
    for h in range(NH):
        dch, po = h // 2, 64 * (h % 2)
        for hf in range(2):          # q-pass split: ctx only [65, 1024] psum
            ctx = ctxps.tile([DK + 1, HS], F32, tag="ctx", name="ctx")
            atts = {}
            LAG = 6   # emit PV L chunks behind scores: next L scores outrank it
            for cc in range(KC + LAG):
                if cc < KC:
                    c = cc
                    if h == 0 and hf == 0:
                        vproj_grain(c)
                    scs = ps.tile([128, HS], F32, tag="ps", name="ps")
                    for qc in range(2):
                        nc.tensor.matmul(
                            scs[:, qc * 512:(qc + 1) * 512],
                            kt[dch][po:po + DK, c * 128:(c + 1) * 128],
                            qt[dch][po:po + DK,
                                    hf * HS + qc * 512:hf * HS + (qc + 1) * 512],
                            start=True, stop=True)
                    att = attnp.tile([128, HS], BF16, tag="attn", name="attn")
                    nc.scalar.activation(att[:], scs[:],
                                         mybir.ActivationFunctionType.Exp,
                                         scale=0.125)
                    atts[c] = att
                if cc >= LAG:
                    c = cc - LAG
                    att = atts.pop(c)
                    for qc in range(2):
                        nc.tensor.matmul(
                            ctx[:, qc * 512:(qc + 1) * 512],
                            vaug[:, c, h, :],
                            att[:, qc * 512:(qc + 1) * 512],
                            start=(c == 0), stop=(c == KC - 1))
                if pending and cc % 2 == 1:
                    outproj_grain(*pending.popleft())
            cp = zp.tile([DK + 1, HS], F32, tag="cp", name="cp")
            nc.vector.tensor_copy(cp[:], ctx[:])  # frees the ctx psum slot fast
            zr = zp.tile([1, HS], F32, tag="zr", name="zr")
            nc.vector.reciprocal(zr[:], cp[DK:DK + 1, :])
            bc = zp.tile([DK, HS], F32, tag="bc", name="bc")
            nc.gpsimd.partition_broadcast(bc[:], zr[:])
            nc.vector.tensor_mul(ctxn[dch][po:po + DK, hf * HS:(hf + 1) * HS],
                                 cp[0:DK, :], bc[:])
            if h == 3 and hf == 0:
                # ctxn[1][:, 0:HS] complete -> its 8 pairs can go
                pending.extend((1, oc, 0, outT1, 0) for oc in range(8))

        if h == 1:
            pending.extend((0, oc, qp, outT0, 0)
                           for oc in range(8) for qp in range(2))
            # deferred dch1 Q/K projections (needed by heads 2/3 only):
            # re-DMA x into fresh tiles (queue is idle now), small psum grains
            for bofs, (key, xdr, outsb) in enumerate(
                    (("q", xq, qt), ("k", xk, kt))):
                wt = wts[key]
                xt2 = []
                for m in range(MT):
                    t = xp.tile([128, S], BF16, tag="x", name="x2")
                    nc.sync.dma_start(t[:], xdr[m * 128:(m + 1) * 128, :])
                    xt2.append(t)
                for hf2 in range(2):
                    for qc in range(2):
                        psum = smallps.tile([128, 512], F32, tag="sm",
                                            name="ps2")
                        for m in range(MT):
                            nc.tensor.matmul(
                                psum[:],
                                wt[m][:, 128:256],
                                xt2[m][:, hf2 * HS + qc * 512:
                                       hf2 * HS + (qc + 1) * 512],
                                start=(m == 0), stop=False)
                        nc.tensor.matmul(
                            psum[:],
                            bt[:, bofs * DG + 128:bofs * DG + 256],
                            ones[:, hf2 * HS + qc * 512:
                                 hf2 * HS + (qc + 1) * 512],
                            start=False, stop=True)
                        nc.vector.tensor_copy(
                            outsb[1][:, hf2 * HS + qc * 512:
                                     hf2 * HS + (qc + 1) * 512], psum[:])
    # tail: whatever pairs remain, plus the dch1 upper-half pass
    pending.extend((1, oc, 1, outT1, oc) for oc in range(8))
    for g in pending:
        outproj_grain(*g)


def build_nc(reps=1):
    nc = bacc.Bacc("TRN2", target_bir_lowering=False)
    dram = (
        nc.dram_tensor("xq", [D, S], BF16, kind="ExternalInput"),
        nc.dram_tensor("xk", [D, S], BF16, kind="ExternalInput"),
        nc.dram_tensor("xv", [D, S], BF16, kind="ExternalInput"),
        nc.dram_tensor("wq", [D, DG], BF16, kind="ExternalInput"),
        nc.dram_tensor("wk", [D, DG], BF16, kind="ExternalInput"),
        nc.dram_tensor("wv", [D, DG], BF16, kind="ExternalInput"),
        nc.dram_tensor("bT", [1, 3 * DG], BF16, kind="ExternalInput"),
        nc.dram_tensor("wo", [DG, D], BF16, kind="ExternalInput"),
        nc.dram_tensor("outT0", [D, S], BF16, kind="ExternalOutput"),
        nc.dram_tensor("outT1", [D, S], BF16, kind="ExternalOutput"),
    )

    with tile.TileContext(nc) as tc:
        with (
            tc.tile_pool(name="persist", bufs=1) as persist,
            tc.tile_pool(name="xp", bufs=16) as xp,
            tc.tile_pool(name="wp", bufs=26) as wp,
            tc.tile_pool(name="wop", bufs=2) as wop,
            tc.tile_pool(name="attnp", bufs=20) as attnp,
            tc.tile_pool(name="zp", bufs=2) as zp,
            tc.tile_pool(name="outp", bufs=6) as outp,
            tc.tile_pool(name="ps", bufs=2, space="PSUM") as ps,
            tc.tile_pool(name="ctxps", bufs=1, space="PSUM") as ctxps,
            tc.tile_pool(name="smallps", bufs=2, space="PSUM") as smallps,
        ):
            pools = (persist, xp, wp, wop, attnp, zp, outp, ps, ctxps, smallps)
            if reps == 1:
                _emit(nc, pools, dram)
            else:
                with tc.For_i(0, reps, 1):
                    _emit(nc, pools, dram)
    nc.compile()
    return nc


def make_in_maps(query, key, value, Wq, bq, Wk, bk, Wv, bv, Wo, bo):
    bf = ml_dtypes.bfloat16
    query, key, value = (np.asarray(a, np.float32) for a in (query, key, value))
    Wq, bq, Wk, bk, Wv, bv, Wo, bo = (
        np.asarray(a, np.float32) for a in (Wq, bq, Wk, bk, Wv, bv, Wo, bo))
    in_maps = []
    for c in range(N_CORES):
        b, g = divmod(c, 4)
        sl = slice(g * DG, (g + 1) * DG)

        def xa(x):
            return np.ascontiguousarray(x[b].T).astype(bf)

        def wa(W):
            return np.ascontiguousarray(W[sl, :].T).astype(bf)

        in_maps.append({
            "xq": xa(query), "xk": xa(key), "xv": xa(value),
            "wq": wa(Wq), "wk": wa(Wk), "wv": wa(Wv),
            "bT": np.concatenate([bq[sl], bk[sl], bv[sl]])[None, :].astype(bf),
            "wo": np.ascontiguousarray(Wo[:, sl].T).astype(bf),
        })
    return in_maps


_NC_CACHE = {}


def kernel(query, key, value, Wq, bq, Wk, bk, Wv, bv, Wo, bo):
    in_maps = make_in_maps(query, key, value, Wq, bq, Wk, bk, Wv, bv, Wo, bo)
    if 1 not in _NC_CACHE:
        _NC_CACHE[1] = build_nc(1)
    nc = _NC_CACHE[1]
    res = run_bass_kernel_spmd(nc, in_maps, core_ids=list(range(N_CORES)))
    out = np.zeros((2, S, D), np.float32)
    for c in range(N_CORES):
        b = c // 4
        out[b] += np.asarray(res.results[c]["outT0"], np.float32).T
        out[b] += np.asarray(res.results[c]["outT1"], np.float32).T
    out += np.asarray(bo, np.float32)[None, None, :]
    return out



# revision 9
# speedup vs baseline: 1.1206x; 1.1206x over previous
"""Multi-head attention (B=2, S=2048, D=1024, H=16) on 8 TRN2 NeuronCores.

Sharding: core c -> (batch b = c//4, head-group g = c%4). Each core computes
the attention output restricted to its batch and its 4 heads (a 256-wide
slice of the model dim), including the row-parallel output projection
partial product. Host sums the 4 partials per batch and adds bo.

Device-side layouts (everything transposed so no on-device transposes are
needed):
  xq/xk/xv  bf16 [1024, 2048]  = x[b].T
  wq/wk/wv  bf16 [1024, 256]   = W[g-slice, :].T
  bqk       f32  [128, 4]      = per-partition bias cols (bq d0,d1, bk d0,d1)
  bvr       f32  [1, 256]      = bv row (broadcast on device)
  wo        bf16 [256, 1024]   = Wo[:, g-slice].T
  outT      bf16 [1024, 2048]  = (Wo_g @ ctxn_g^T) partial, host transposes

Pipeline per core, paced by a 64-slot schedule (4 phases x 16 k-chunks,
phase = (dch pair of heads, q-half)):
  Q^T,K^T = W x^T              (PE, psum accum; bias added by DVE eviction)
  V       = x^T-stationary     (natural [s, d] layout; bias via DVE add)
  slot (p=(dch,hf), c):
    scores^T[k,q] two heads CONCURRENTLY (row-tiled PE: head A rows 0:64,
        head B rows 64:128 -> separate psum banks)
    att = exp(scores/8)        (ACT, psum->sbuf bf16; ACT is the pacer)
  PV units (head,hf,c) consumed from a lagged queue: ctx_aug^T[65,q] +=
        V_aug^T-stationary @ att  (psum accum, ones row = softmax Z)
  ctxn^T = ctx^T * recip(Z)    (DVE copy + recip + gpsimd bcast + DVE mul)
  outT[oc] = wo-stationary @ ctxn^T  (PE grains interleaved into slots)
"""

import numpy as np
import ml_dtypes
from collections import defaultdict, deque

from concourse import bacc, tile, mybir
from concourse.bass_utils import run_bass_kernel_spmd

BF16 = mybir.dt.bfloat16
F32 = mybir.dt.float32

S = 2048      # sequence length
D = 1024      # model dim
DG = 256      # per-core head-group width (4 heads x 64)
DK = 64       # head dim
NH = 4        # heads per core
MT = 8        # model-dim contraction tiles (1024 / 128)
HS = S // 2   # q-half width
KC = 16       # k chunks of 128
N_CORES = 8

PV_START = 10   # first slot that consumes PV work
AUX_RATE = 3    # max aux closures popped per slot


def _emit(nc, pools, dram):
    persist, xp, wp, wop, attnp, zp, outp, ps, ctxps, smallps = pools
    xq, xk, xv, wq, wk, wv, bqk, bvr, wo, outT0, outT1 = dram

    # ---------------- persistent tiles ----------------
    qt = [persist.tile([128, S], BF16, tag=f"qt{i}", name=f"qt{i}") for i in range(2)]
    kt = [persist.tile([128, S], BF16, tag=f"kt{i}", name=f"kt{i}") for i in range(2)]
    ctxn = [persist.tile([128, S], BF16, tag=f"ctxn{i}", name=f"ctxn{i}")
            for i in range(2)]
    vaug = persist.tile([128, KC, NH, DK + 1], BF16, tag="vaug", name="vaug")
    nc.vector.memset(vaug[:, :, :, DK:DK + 1], 1.0)

    bqkt = persist.tile([128, 4], F32, tag="bqkt", name="bqkt")
    nc.sync.dma_start(bqkt[:], bqk[:])
    bvrow = persist.tile([1, NH * DK], F32, tag="bvrow", name="bvrow")
    nc.sync.dma_start(bvrow[:], bvr[:])
    bvb = persist.tile([128, NH, DK], F32, tag="bvb", name="bvb")
    nc.gpsimd.partition_broadcast(bvb[:], bvrow[:])

    # ---------------- weight + x DMAs (front of queue) ----------------
    wts = {}
    for key, wdr in (("q", wq), ("k", wk)):
        wt = []
        for m in range(MT):
            t = wp.tile([128, DG], BF16, tag="w", name="w")
            nc.sync.dma_start(t[:], wdr[m * 128:(m + 1) * 128, :])
            wt.append(t)
        wts[key] = wt

    xts = {}
    for key, xdr in (("q", xq), ("k", xk)):
        xt = [xp.tile([128, S], BF16, tag="x", name="x") for _ in range(MT)]
        for m in range(MT):
            nc.sync.dma_start(xt[m][:, 0:HS], xdr[m * 128:(m + 1) * 128, 0:HS])
        xts[key] = xt

    # ---------------- dch0 projections, hf0 (pre-slot work) ----------------
    BCOL = {"q": 0, "k": 2}

    def proj_big(key, dch, hf):
        """[128, 1024] projection pass in the big psum pool."""
        outsb = qt if key == "q" else kt
        psum = ps.tile([128, HS], F32, tag="ps", name="ps")
        for m in range(MT):
            for qc in range(2):
                nc.tensor.matmul(
                    psum[:, qc * 512:(qc + 1) * 512],
                    wts[key][m][:, dch * 128:(dch + 1) * 128],
                    xts[key][m][:, hf * HS + qc * 512:hf * HS + (qc + 1) * 512],
                    start=(m == 0), stop=(m == MT - 1))
        nc.vector.tensor_scalar_add(
            outsb[dch][:, hf * HS:(hf + 1) * HS], psum[:],
            bqkt[:, BCOL[key] + dch:BCOL[key] + dch + 1])

    proj_big("q", 0, 0)
    proj_big("k", 0, 0)

    # xk hf1 next (kt must be FULL-width before scores chunk 8), then wv+xv
    # (vproj feeds PV from slot ~10), then xq hf1.
    for m in range(MT):
        nc.sync.dma_start(xts["k"][m][:, HS:S], xk[m * 128:(m + 1) * 128, HS:S])
    wvt = []
    for m in range(MT):
        t = wp.tile([128, DG], BF16, tag="w", name="w")
        nc.sync.dma_start(t[:], wv[m * 128:(m + 1) * 128, :])
        wvt.append(t)
    xvt = [xp.tile([128, S], BF16, tag="x", name="xv") for _ in range(MT)]
    for m in range(MT):
        nc.sync.dma_start(xvt[m][:], xv[m * 128:(m + 1) * 128, :])
    for m in range(MT):
        nc.sync.dma_start(xts["q"][m][:, HS:S], xq[m * 128:(m + 1) * 128, HS:S])
    wot = []
    for dch in range(2):
        t = wop.tile([128, D], BF16, tag="wo", name="wo")
        nc.sync.dma_start(t[:], wo[dch * 128:(dch + 1) * 128, :])
        wot.append(t)

    # ---------------- aux work (fills PE gaps in early slots) ----------------
    def proj_small(key, dch, hf, qc, xtiles):
        outsb = qt if key == "q" else kt
        psum = smallps.tile([128, 512], F32, tag="sm", name="ps2")
        for m in range(MT):
            nc.tensor.matmul(
                psum[:],
                wts[key][m][:, dch * 128:(dch + 1) * 128],
                xtiles[m][:, hf * HS + qc * 512:hf * HS + (qc + 1) * 512],
                start=(m == 0), stop=(m == MT - 1))
        nc.vector.tensor_scalar_add(
            outsb[dch][:, hf * HS + qc * 512:hf * HS + (qc + 1) * 512],
            psum[:], bqkt[:, BCOL[key] + dch:BCOL[key] + dch + 1])

    def vproj_grain(sc):
        vps = smallps.tile([128, NH, DK], F32, tag="sm", name="vps")
        for m in range(MT):
            nc.tensor.matmul(
                vps[:, :, :],
                xvt[m][:, sc * 128:(sc + 1) * 128],
                wvt[m][:],
                start=(m == 0), stop=(m == MT - 1))
        nc.vector.tensor_add(vaug[:, sc, :, 0:DK], vps[:, :, :], bvb[:])

    aux = []  # (min_slot, closure) sorted by min_slot
    for qc in range(2):  # kt full width needed by scores chunk 8
        aux.append((3 + qc, lambda qc=qc: proj_small("k", 0, 1, qc, xts["k"])))
    for sc in range(KC):
        aux.append((8 + sc // 2, lambda sc=sc: vproj_grain(sc)))
    for qc in range(2):  # qt hf1 needed by phase 1 (slot 16)
        aux.append((11 + qc, lambda qc=qc: proj_small("q", 0, 1, qc, xts["q"])))
    for hf in range(2):  # dch1: kt[1] full by slot 32/40, qt[1] by 32/48
        for qc in range(2):
            aux.append((14 + 2 * hf + 2 * qc,
                        lambda hf=hf, qc=qc: proj_small("k", 1, hf, qc, xts["k"])))
            aux.append((15 + 2 * hf + 2 * qc,
                        lambda hf=hf, qc=qc: proj_small("q", 1, hf, qc, xts["q"])))
    aux.sort(key=lambda t: t[0])
    aux = deque(aux)

    # ---------------- slot schedules ----------------
    # phases p=0..3: (dch, hf) = (p//2, p%2); groups g=0..7: head h = 2*(g//4...)
    # group g -> phase p_g = g//2, head-in-pair a = g%2, h = 2*(g//4)...
    def group_head_hf(g):
        p = g // 2
        dch, hf = p // 2, p % 2
        h = 2 * dch + (g % 2)
        return h, hf

    pv_sched = defaultdict(list)   # slot -> [(g, c)]
    tail_pv = []
    for g in range(8):
        for c in range(KC):
            s = PV_START + 8 * g + (c // 2 if g < 7 else c // 2 - 2)
            if g == 7:
                s = max(s, 48 + c + 1)  # att of phase3 chunk c lands at slot 48+c
            if s <= 63:
                pv_sched[s].append((g, c))
            else:
                tail_pv.append((g, c))

    # outproj grains (dch, oc, qp): ready after ctxn[dch] qp-half (group 2*...)
    op_sched = defaultdict(list)
    tail_op = []
    for dch, qp, s0 in ((0, 0, 30), (0, 1, 46), (1, 0, 60)):
        for oc in range(8):
            s = s0 + (oc if qp == 0 and dch == 0 else
                      oc if dch == 0 else oc // 2)
            if s <= 63:
                op_sched[s].append((dch, oc, qp))
            else:
                tail_op.append((dch, oc, qp))
    for oc in range(8):
        tail_op.append((1, oc, 1))

    # ---------------- emission helpers ----------------
    atts = {}      # (h, hf, c) -> att tile
    ctx_tiles = {}  # g -> psum tile

    def emit_scores_exp(dch, hf, c):
        kt_d, qt_d = kt[dch], qt[dch]
        for a in range(2):  # head-in-pair; row-tiled concurrent on PE
            psc = ps.tile([128, HS], F32, tag="ps", name="ps")
            po = 64 * a
            for qc in range(2):
                nc.tensor.matmul(
                    psc[:, qc * 512:(qc + 1) * 512],
                    kt_d[po:po + DK, c * 128:(c + 1) * 128],
                    qt_d[po:po + DK, hf * HS + qc * 512:hf * HS + (qc + 1) * 512],
                    start=True, stop=True)
            att = attnp.tile([128, HS], BF16, tag="attn", name="attn")
            nc.scalar.activation(att[:], psc[:],
                                 mybir.ActivationFunctionType.Exp, scale=0.125)
            atts[(2 * dch + a, hf, c)] = att

    def emit_pv(g, c):
        h, hf = group_head_hf(g)
        if g not in ctx_tiles:
            ctx_tiles[g] = ctxps.tile([DK + 1, HS], F32, tag="ctx", name="ctx")
        ctx = ctx_tiles[g]
        att = atts.pop((h, hf, c))
        for qc in range(2):
            nc.tensor.matmul(
                ctx[:, qc * 512:(qc + 1) * 512],
                vaug[:, c, h, :],
                att[:, qc * 512:(qc + 1) * 512],
                start=(c == 0), stop=(c == KC - 1))
        if c == KC - 1:
            emit_ctxn(g, ctx)
            del ctx_tiles[g]

    def emit_ctxn(g, ctx):
        h, hf = group_head_hf(g)
        dch, po = h // 2, 64 * (h % 2)
        cp = zp.tile([DK + 1, HS], F32, tag="cp", name="cp")
        nc.vector.tensor_copy(cp[:], ctx[:])  # frees the ctx psum slot fast
        zr = zp.tile([1, HS], F32, tag="zr", name="zr")
        nc.vector.reciprocal(zr[:], cp[DK:DK + 1, :])
        bc = zp.tile([DK, HS], F32, tag="bc", name="bc")
        nc.gpsimd.partition_broadcast(bc[:], zr[:])
        nc.vector.tensor_mul(ctxn[dch][po:po + DK, hf * HS:(hf + 1) * HS],
                             cp[0:DK, :], bc[:])

    def outproj_grain(dch, oc, qp):
        outT = outT0 if dch == 0 else outT1
        osb = outp.tile([128, HS], BF16, tag="out", name="out")
        for j in range(2):
            q4 = 2 * qp + j
            ops = smallps.tile([128, 512], F32, tag="sm", name="ops")
            nc.tensor.matmul(
                ops[:], wot[dch][:, oc * 128:(oc + 1) * 128],
                ctxn[dch][:, q4 * 512:(q4 + 1) * 512],
                start=True, stop=True)
            nc.vector.tensor_copy(osb[:, j * 512:(j + 1) * 512], ops[:])
        nc.sync.dma_start(
            outT[oc * 128:(oc + 1) * 128, qp * HS:(qp + 1) * HS], osb[:])

    # ---------------- main slot loop ----------------
    for slot in range(64):
        p = slot // 16
        dch, hf = p // 2, p % 2
        c = slot % 16
        emit_scores_exp(dch, hf, c)
        for g, cc in pv_sched.get(slot, ()):
            emit_pv(g, cc)
        for grain in op_sched.get(slot, ()):
            outproj_grain(*grain)
        n = 0
        while aux and aux[0][0] <= slot and n < AUX_RATE:
            _, fn = aux.popleft()
            fn()
            n += 1

    # ---------------- tail ----------------
    while aux:
        _, fn = aux.popleft()
        fn()
    for g, cc in tail_pv:
        emit_pv(g, cc)
    for grain in tail_op:
        outproj_grain(*grain)


def build_nc(reps=1):
    nc = bacc.Bacc("TRN2", target_bir_lowering=False)
    dram = (
        nc.dram_tensor("xq", [D, S], BF16, kind="ExternalInput"),
        nc.dram_tensor("xk", [D, S], BF16, kind="ExternalInput"),
        nc.dram_tensor("xv", [D, S], BF16, kind="ExternalInput"),
        nc.dram_tensor("wq", [D, DG], BF16, kind="ExternalInput"),
        nc.dram_tensor("wk", [D, DG], BF16, kind="ExternalInput"),
        nc.dram_tensor("wv", [D, DG], BF16, kind="ExternalInput"),
        nc.dram_tensor("bqk", [128, 4], F32, kind="ExternalInput"),
        nc.dram_tensor("bvr", [1, DG], F32, kind="ExternalInput"),
        nc.dram_tensor("wo", [DG, D], BF16, kind="ExternalInput"),
        nc.dram_tensor("outT0", [D, S], BF16, kind="ExternalOutput"),
        nc.dram_tensor("outT1", [D, S], BF16, kind="ExternalOutput"),
    )

    with tile.TileContext(nc) as tc:
        with (
            tc.tile_pool(name="persist", bufs=1) as persist,
            tc.tile_pool(name="xp", bufs=24) as xp,
            tc.tile_pool(name="wp", bufs=24) as wp,
            tc.tile_pool(name="wop", bufs=2) as wop,
            tc.tile_pool(name="attnp", bufs=20) as attnp,
            tc.tile_pool(name="zp", bufs=1) as zp,
            tc.tile_pool(name="outp", bufs=4) as outp,
            tc.tile_pool(name="ps", bufs=2, space="PSUM") as ps,
            tc.tile_pool(name="ctxps", bufs=1, space="PSUM") as ctxps,
            tc.tile_pool(name="smallps", bufs=2, space="PSUM") as smallps,
        ):
            pools = (persist, xp, wp, wop, attnp, zp, outp, ps, ctxps, smallps)
            if reps == 1:
                _emit(nc, pools, dram)
            else:
                with tc.For_i(0, reps, 1):
                    _emit(nc, pools, dram)
    nc.compile()
    return nc


def make_in_maps(query, key, value, Wq, bq, Wk, bk, Wv, bv, Wo, bo):
    bf = ml_dtypes.bfloat16
    query, key, value = (np.asarray(a, np.float32) for a in (query, key, value))
    Wq, bq, Wk, bk, Wv, bv, Wo, bo = (
        np.asarray(a, np.float32) for a in (Wq, bq, Wk, bk, Wv, bv, Wo, bo))
    in_maps = []
    for c in range(N_CORES):
        b, g = divmod(c, 4)
        sl = slice(g * DG, (g + 1) * DG)

        def xa(x):
            return np.ascontiguousarray(x[b].T).astype(bf)

        def wa(W):
            return np.ascontiguousarray(W[sl, :].T).astype(bf)

        bqs, bks = bq[sl], bk[sl]
        bqk_t = np.stack([bqs[0:128], bqs[128:256],
                          bks[0:128], bks[128:256]], axis=1).astype(np.float32)
        in_maps.append({
            "xq": xa(query), "xk": xa(key), "xv": xa(value),
            "wq": wa(Wq), "wk": wa(Wk), "wv": wa(Wv),
            "bqk": np.ascontiguousarray(bqk_t),
            "bvr": np.ascontiguousarray(bv[sl][None, :].astype(np.float32)),
            "wo": np.ascontiguousarray(Wo[:, sl].T).astype(bf),
        })
    return in_maps


_NC_CACHE = {}


def kernel(query, key, value, Wq, bq, Wk, bk, Wv, bv, Wo, bo):
    in_maps = make_in_maps(query, key, value, Wq, bq, Wk, bk, Wv, bv, Wo, bo)
    if 1 not in _NC_CACHE:
        _NC_CACHE[1] = build_nc(1)
    nc = _NC_CACHE[1]
    res = run_bass_kernel_spmd(nc, in_maps, core_ids=list(range(N_CORES)))
    out = np.zeros((2, S, D), np.float32)
    for c in range(N_CORES):
        b = c // 4
        out[b] += np.asarray(res.results[c]["outT0"], np.float32).T
        out[b] += np.asarray(res.results[c]["outT1"], np.float32).T
    out += np.asarray(bo, np.float32)[None, None, :]
    return out


# revision 25
# speedup vs baseline: 1.1473x; 1.0239x over previous
"""Multi-head attention (B=2, S=2048, D=1024, H=16) on 8 TRN2 NeuronCores.

Sharding: core c -> (batch b = c//4, head-group g = c%4). Each core computes
the attention output restricted to its batch and its 4 heads (a 256-wide
slice of the model dim), including the row-parallel output projection
partial product. Host sums the 4 partials per batch and adds bo.

Device-side layouts (everything transposed so no on-device transposes are
needed):
  xq/xk/xv  bf16 [1024, 2048]  = x[b].T
  wq/wk/wv  bf16 [1024, 256]   = W[g-slice, :].T
  bqk       f32  [128, 4]      = per-partition bias cols (bq d0,d1, bk d0,d1)
  bvr       f32  [1, 256]      = bv row (broadcast on device)
  wo        bf16 [256, 1024]   = Wo[:, g-slice].T
  outT      bf16 [1024, 2048]  = (Wo_g @ ctxn_g^T) partial, host transposes

Pipeline per core, paced by a 64-slot schedule (4 phases x 16 k-chunks,
phase = (dch pair of heads, q-half)):
  Q^T,K^T = W x^T              (PE, psum accum; bias added by DVE eviction)
  V       = x^T-stationary     (natural [s, d] layout; bias via DVE add)
  slot (p=(dch,hf), c):
    scores^T[k,q] two heads CONCURRENTLY (row-tiled PE: head A rows 0:64,
        head B rows 64:128 -> separate psum banks)
    att = exp(scores/8)        (ACT, psum->sbuf bf16; ACT is the pacer)
  PV units (head,hf,c) consumed from a lagged queue: ctx_aug^T[65,q] +=
        V_aug^T-stationary @ att  (psum accum, ones row = softmax Z)
  ctxn^T = ctx^T * recip(Z)    (DVE copy + recip + gpsimd bcast + DVE mul)
  outT[oc] = wo-stationary @ ctxn^T  (PE grains interleaved into slots)
"""

import numpy as np
import ml_dtypes
from collections import defaultdict, deque

from concourse import bacc, tile, mybir
from concourse.bass_utils import run_bass_kernel_spmd

BF16 = mybir.dt.bfloat16
F32 = mybir.dt.float32

S = 2048      # sequence length
D = 1024      # model dim
DG = 256      # per-core head-group width (4 heads x 64)
DK = 64       # head dim
NH = 4        # heads per core
MT = 8        # model-dim contraction tiles (1024 / 128)
HS = S // 2   # q-half width
KC = 16       # k chunks of 128
N_CORES = 8

PV_START = 10   # first slot that consumes PV work
AUX_RATE = 3    # max aux closures popped per slot


def _emit(nc, pools, dram):
    persist, xp, wp, wop, attnp, zp, outp, ps, ctxps, smallps = pools
    xq, xk, xv, wq, wk, wv, bqk, bvr, wo, outT0, outT1 = dram

    # ---------------- persistent tiles ----------------
    qt = [persist.tile([128, S], BF16, tag=f"qt{i}", name=f"qt{i}") for i in range(2)]
    kt = [persist.tile([128, S], BF16, tag=f"kt{i}", name=f"kt{i}") for i in range(2)]
    ctxn = [persist.tile([128, S], BF16, tag=f"ctxn{i}", name=f"ctxn{i}")
            for i in range(2)]
    vaug = persist.tile([128, KC, NH, DK + 1], BF16, tag="vaug", name="vaug")
    nc.vector.memset(vaug[:, :, :, DK:DK + 1], 1.0)

    # ---------------- weight + x DMAs (front of queue) ----------------
    # Single 3D-AP DMAs: dram [(m p), n] -> sbuf [p, m, n]. Keeps the SP
    # issue queue short so the first projections start ASAP. K-side first
    # (smaller first bite: 512 k-positions suffice for score chunks 0-3).
    def dram3(t):
        return t[:].rearrange("(m p) n -> p m n", p=128)

    wts = {}
    wts["q"] = wp.tile([128, MT, DG], BF16, tag="wq", name="wqt")
    nc.sync.dma_start(wts["q"][:], dram3(wq))
    xts = {}
    xts["q"] = xp.tile([128, MT, S], BF16, tag="xq", name="xqt")
    nc.sync.dma_start(xts["q"][:, :, 0:HS], dram3(xq)[:, :, 0:HS])
    bqkt = persist.tile([128, 4], F32, tag="bqkt", name="bqkt")
    nc.sync.dma_start(bqkt[:], bqk[:])
    bvrow = persist.tile([1, NH * DK], F32, tag="bvrow", name="bvrow")
    nc.sync.dma_start(bvrow[:], bvr[:])
    bvb = persist.tile([128, NH, DK], F32, tag="bvb", name="bvb")
    nc.gpsimd.partition_broadcast(bvb[:], bvrow[:])
    wts["k"] = wp.tile([128, MT, DG], BF16, tag="wk", name="wkt")
    nc.sync.dma_start(wts["k"][:], dram3(wk))
    xts["k"] = xp.tile([128, MT, S], BF16, tag="xk", name="xkt")
    nc.sync.dma_start(xts["k"][:, :, 0:512], dram3(xk)[:, :, 0:512])

    # ---------------- dch0 projections, hf0 (pre-slot work) ----------------
    BCOL = {"q": 0, "k": 2}

    def proj_big(key, dch, hf):
        """[128, 1024] projection pass in the big psum pool."""
        outsb = qt if key == "q" else kt
        psum = ps.tile([128, HS], F32, tag="ps", name="ps")
        for m in range(MT):
            for qc in range(2):
                nc.tensor.matmul(
                    psum[:, qc * 512:(qc + 1) * 512],
                    wts[key][:, m, dch * 128:(dch + 1) * 128],
                    xts[key][:, m, hf * HS + qc * 512:hf * HS + (qc + 1) * 512],
                    start=(m == 0), stop=(m == MT - 1))
        nc.vector.tensor_scalar_add(
            outsb[dch][:, hf * HS:(hf + 1) * HS], psum[:],
            bqkt[:, BCOL[key] + dch:BCOL[key] + dch + 1])

    # ---------------- aux work (fills PE gaps in early slots) ----------------
    def proj_small(key, dch, hf, qc, xtiles):
        outsb = qt if key == "q" else kt
        psum = smallps.tile([128, 512], F32, tag="sm", name="ps2")
        for m in range(MT):
            nc.tensor.matmul(
                psum[:],
                wts[key][:, m, dch * 128:(dch + 1) * 128],
                xtiles[:, m, hf * HS + qc * 512:hf * HS + (qc + 1) * 512],
                start=(m == 0), stop=(m == MT - 1))
        nc.vector.tensor_scalar_add(
            outsb[dch][:, hf * HS + qc * 512:hf * HS + (qc + 1) * 512],
            psum[:], bqkt[:, BCOL[key] + dch:BCOL[key] + dch + 1])

    def vproj_grain(sc):
        vps = smallps.tile([128, NH, DK], F32, tag="sm", name="vps")
        for m in range(MT):
            nc.tensor.matmul(
                vps[:, :, :],
                xvt[:, m, sc * 128:(sc + 1) * 128],
                wvt[:, m, :],
                start=(m == 0), stop=(m == MT - 1))
        nc.vector.tensor_add(vaug[:, sc, :, 0:DK], vps[:, :, :], bvb[:])

    proj_big("q", 0, 0)
    proj_small("k", 0, 0, 0, xts["k"])  # kt[0][:, 0:512]: score chunks 0-3

    # rest of the DMA queue, in need order
    nc.sync.dma_start(xts["k"][:, :, 512:HS], dram3(xk)[:, :, 512:HS])
    nc.sync.dma_start(xts["k"][:, :, HS:S], dram3(xk)[:, :, HS:S])
    wvt = wp.tile([128, MT, DG], BF16, tag="wv", name="wv")
    nc.sync.dma_start(wvt[:], dram3(wv))
    xvt = xp.tile([128, MT, S], BF16, tag="xv", name="xv")
    nc.sync.dma_start(xvt[:], dram3(xv))
    nc.sync.dma_start(xts["q"][:, :, HS:S], dram3(xq)[:, :, HS:S])
    wot = wop.tile([128, 2, D], BF16, tag="wo", name="wo")
    nc.sync.dma_start(wot[:], dram3(wo))

    aux = []  # (min_slot, closure) sorted by min_slot
    # kt[0] progressive widening: cols 512:1024 by chunk 4, hf1 by chunk 8
    aux.append((1, lambda: proj_small("k", 0, 0, 1, xts["k"])))
    aux.append((4, lambda: proj_small("k", 0, 1, 0, xts["k"])))
    aux.append((5, lambda: proj_small("k", 0, 1, 1, xts["k"])))
    for sc in range(KC):
        aux.append((9 + sc // 2, lambda sc=sc: vproj_grain(sc)))
    for qc in range(2):  # qt hf1 needed by phase 1 (slot 16)
        aux.append((13 + qc, lambda qc=qc: proj_small("q", 0, 1, qc, xts["q"])))
    for i, (key, hf) in enumerate((("k", 0), ("q", 0), ("k", 1), ("q", 1))):
        for qc in range(2):  # dch1: kt[1] by slot 32/40, qt[1] by 32/48
            aux.append((24 + 3 * i + qc,
                        lambda key=key, hf=hf, qc=qc:
                        proj_small(key, 1, hf, qc, xts[key])))
    aux.sort(key=lambda t: t[0])
    aux = deque(aux)

    # ---------------- slot schedules ----------------
    # phases p=0..3: (dch, hf) = (p//2, p%2); groups g=0..7: head h = 2*(g//4...)
    # group g -> phase p_g = g//2, head-in-pair a = g%2, h = 2*(g//4)...
    def group_head_hf(g):
        p = g // 2
        dch, hf = p // 2, p % 2
        h = 2 * dch + (g % 2)
        return h, hf

    pv_sched = defaultdict(list)   # slot -> [(g, c)]
    tail_pv = []
    for g in range(8):
        for c in range(KC):
            s = PV_START + 8 * g + (c // 2 if g < 7 else c // 2 - 2)
            if g == 7:
                s = max(s, 48 + c + 1)  # att of phase3 chunk c lands at slot 48+c
            if s <= 63:
                pv_sched[s].append((g, c))
            else:
                tail_pv.append((g, c))

    # outproj grains (dch, oc, qp): ready after ctxn[dch] qp-half (group 2*...)
    op_sched = defaultdict(list)
    tail_op = []
    for dch, qp, s0 in ((0, 0, 30), (0, 1, 46), (1, 0, 60)):
        for oc in range(8):
            s = s0 + (oc if qp == 0 and dch == 0 else
                      oc if dch == 0 else oc // 2)
            if s <= 63:
                op_sched[s].append((dch, oc, qp))
            else:
                tail_op.append((dch, oc, qp))
    for oc in range(8):
        tail_op.append((1, oc, 1))

    # ---------------- emission helpers ----------------
    atts = {}      # (h, hf, c) -> att tile
    ctx_tiles = {}  # g -> psum tile

    def emit_scores_exp(dch, hf, c):
        kt_d, qt_d = kt[dch], qt[dch]
        for a in range(2):  # head-in-pair; row-tiled concurrent on PE
            psc = ps.tile([128, HS], F32, tag="ps", name="ps")
            po = 64 * a
            for qc in range(2):
                nc.tensor.matmul(
                    psc[:, qc * 512:(qc + 1) * 512],
                    kt_d[po:po + DK, c * 128:(c + 1) * 128],
                    qt_d[po:po + DK, hf * HS + qc * 512:hf * HS + (qc + 1) * 512],
                    start=True, stop=True)
            att = attnp.tile([128, HS], BF16, tag="attn", name="attn")
            nc.scalar.activation(att[:], psc[:],
                                 mybir.ActivationFunctionType.Exp, scale=0.125)
            atts[(2 * dch + a, hf, c)] = att

    def emit_pv(g, c):
        h, hf = group_head_hf(g)
        if g not in ctx_tiles:
            ctx_tiles[g] = ctxps.tile([DK + 1, HS], F32, tag="ctx", name="ctx")
        ctx = ctx_tiles[g]
        att = atts.pop((h, hf, c))
        for qc in range(2):
            nc.tensor.matmul(
                ctx[:, qc * 512:(qc + 1) * 512],
                vaug[:, c, h, :],
                att[:, qc * 512:(qc + 1) * 512],
                start=(c == 0), stop=(c == KC - 1))
        if c == KC - 1:
            emit_ctxn(g, ctx)
            del ctx_tiles[g]

    def emit_ctxn(g, ctx):
        h, hf = group_head_hf(g)
        dch, po = h // 2, 64 * (h % 2)
        cp = zp.tile([DK + 1, HS], F32, tag="cp", name="cp")
        nc.vector.tensor_copy(cp[:], ctx[:])  # frees the ctx psum slot fast
        zr = zp.tile([1, HS], F32, tag="zr", name="zr")
        nc.vector.reciprocal(zr[:], cp[DK:DK + 1, :])
        bc = zp.tile([DK, HS], F32, tag="bc", name="bc")
        nc.gpsimd.partition_broadcast(bc[:], zr[:])
        nc.vector.tensor_mul(ctxn[dch][po:po + DK, hf * HS:(hf + 1) * HS],
                             cp[0:DK, :], bc[:])

    def outproj_grain(dch, oc, qp, act_ev=False):
        outT = outT0 if dch == 0 else outT1
        osb = outp.tile([128, HS], BF16, tag="out", name="out")
        for j in range(2):
            q4 = 2 * qp + j
            ops = smallps.tile([128, 512], F32, tag="sm", name="ops")
            nc.tensor.matmul(
                ops[:], wot[:, dch, oc * 128:(oc + 1) * 128],
                ctxn[dch][:, q4 * 512:(q4 + 1) * 512],
                start=True, stop=True)
            if act_ev and j == 0:  # ACT is idle in the tail; split evictions
                nc.scalar.copy(osb[:, j * 512:(j + 1) * 512], ops[:])
            else:
                nc.vector.tensor_copy(osb[:, j * 512:(j + 1) * 512], ops[:])
        nc.sync.dma_start(
            outT[oc * 128:(oc + 1) * 128, qp * HS:(qp + 1) * HS], osb[:])

    # ---------------- main slot loop ----------------
    for slot in range(64):
        p = slot // 16
        dch, hf = p // 2, p % 2
        c = slot % 16
        emit_scores_exp(dch, hf, c)
        for g, cc in pv_sched.get(slot, ()):
            emit_pv(g, cc)
        for grain in op_sched.get(slot, ()):
            outproj_grain(*grain)
        n = 0
        while aux and aux[0][0] <= slot and n < AUX_RATE:
            _, fn = aux.popleft()
            fn()
            n += 1

    # ---------------- tail ----------------
    while aux:
        _, fn = aux.popleft()
        fn()
    for g, cc in tail_pv:
        if g == 7:
            continue  # g7 handled below with qc-split chains
        emit_pv(g, cc)
    # g7 (last head, hf1): all qc0 matmuls, then the qc0 normalization chain
    # overlapping the qc1 matmuls; outproj j=0 grains can start after the
    # qc0 chain.
    h7, hf7 = group_head_hf(7)
    dch7, po7 = h7 // 2, 64 * (h7 % 2)
    ctx7 = ctxps.tile([DK + 1, HS], F32, tag="ctx", name="ctx7")
    cp7 = zp.tile([DK + 1, HS], F32, tag="cp", name="cp7")
    zr7 = zp.tile([1, HS], F32, tag="zr", name="zr7")
    bc7 = zp.tile([DK, HS], F32, tag="bc", name="bc7")
    g7c = [c for g, c in tail_pv if g == 7]
    for qc in range(2):
        lo, hi = qc * 512, (qc + 1) * 512
        for c in g7c:
            att = atts[(h7, hf7, c)] if qc == 0 else atts.pop((h7, hf7, c))
            nc.tensor.matmul(ctx7[:, lo:hi], vaug[:, c, h7, :], att[:, lo:hi],
                             start=(c == 0), stop=(c == KC - 1))
        nc.vector.tensor_copy(cp7[:, lo:hi], ctx7[:, lo:hi])
        nc.vector.reciprocal(zr7[:, lo:hi], cp7[DK:DK + 1, lo:hi])
        nc.gpsimd.partition_broadcast(bc7[:, lo:hi], zr7[:, lo:hi])
        nc.vector.tensor_mul(
            ctxn[dch7][po7:po7 + DK, hf7 * HS + lo:hf7 * HS + hi],
            cp7[0:DK, lo:hi], bc7[:, lo:hi])
    for grain in tail_op:
        outproj_grain(*grain, act_ev=True)


def build_nc(reps=1):
    nc = bacc.Bacc("TRN2", target_bir_lowering=False)
    dram = (
        nc.dram_tensor("xq", [D, S], BF16, kind="ExternalInput"),
        nc.dram_tensor("xk", [D, S], BF16, kind="ExternalInput"),
        nc.dram_tensor("xv", [D, S], BF16, kind="ExternalInput"),
        nc.dram_tensor("wq", [D, DG], BF16, kind="ExternalInput"),
        nc.dram_tensor("wk", [D, DG], BF16, kind="ExternalInput"),
        nc.dram_tensor("wv", [D, DG], BF16, kind="ExternalInput"),
        nc.dram_tensor("bqk", [128, 4], F32, kind="ExternalInput"),
        nc.dram_tensor("bvr", [1, DG], F32, kind="ExternalInput"),
        nc.dram_tensor("wo", [DG, D], BF16, kind="ExternalInput"),
        nc.dram_tensor("outT0", [D, S], BF16, kind="ExternalOutput"),
        nc.dram_tensor("outT1", [D, S], BF16, kind="ExternalOutput"),
    )

    with tile.TileContext(nc) as tc:
        with (
            tc.tile_pool(name="persist", bufs=1) as persist,
            tc.tile_pool(name="xp", bufs=1) as xp,
            tc.tile_pool(name="wp", bufs=1) as wp,
            tc.tile_pool(name="wop", bufs=2) as wop,
            tc.tile_pool(name="attnp", bufs=19) as attnp,
            tc.tile_pool(name="zp", bufs=1) as zp,
            tc.tile_pool(name="outp", bufs=3) as outp,
            tc.tile_pool(name="ps", bufs=2, space="PSUM") as ps,
            tc.tile_pool(name="ctxps", bufs=1, space="PSUM") as ctxps,
            tc.tile_pool(name="smallps", bufs=2, space="PSUM") as smallps,
        ):
            pools = (persist, xp, wp, wop, attnp, zp, outp, ps, ctxps, smallps)
            if reps == 1:
                _emit(nc, pools, dram)
            else:
                with tc.For_i(0, reps, 1):
                    _emit(nc, pools, dram)
    nc.compile()
    return nc


def make_in_maps(query, key, value, Wq, bq, Wk, bk, Wv, bv, Wo, bo):
    bf = ml_dtypes.bfloat16
    query, key, value = (np.asarray(a, np.float32) for a in (query, key, value))
    Wq, bq, Wk, bk, Wv, bv, Wo, bo = (
        np.asarray(a, np.float32) for a in (Wq, bq, Wk, bk, Wv, bv, Wo, bo))
    in_maps = []
    for c in range(N_CORES):
        b, g = divmod(c, 4)
        sl = slice(g * DG, (g + 1) * DG)

        def xa(x):
            return np.ascontiguousarray(x[b].T).astype(bf)

        def wa(W):
            return np.ascontiguousarray(W[sl, :].T).astype(bf)

        bqs, bks = bq[sl], bk[sl]
        bqk_t = np.stack([bqs[0:128], bqs[128:256],
                          bks[0:128], bks[128:256]], axis=1).astype(np.float32)
        in_maps.append({
            "xq": xa(query), "xk": xa(key), "xv": xa(value),
            "wq": wa(Wq), "wk": wa(Wk), "wv": wa(Wv),
            "bqk": np.ascontiguousarray(bqk_t),
            "bvr": np.ascontiguousarray(bv[sl][None, :].astype(np.float32)),
            "wo": np.ascontiguousarray(Wo[:, sl].T).astype(bf),
        })
    return in_maps


_NC_CACHE = {}


def kernel(query, key, value, Wq, bq, Wk, bk, Wv, bv, Wo, bo):
    in_maps = make_in_maps(query, key, value, Wq, bq, Wk, bk, Wv, bv, Wo, bo)
    if 1 not in _NC_CACHE:
        _NC_CACHE[1] = build_nc(1)
    nc = _NC_CACHE[1]
    res = run_bass_kernel_spmd(nc, in_maps, core_ids=list(range(N_CORES)))
    out = np.zeros((2, S, D), np.float32)
    for c in range(N_CORES):
        b = c // 4
        out[b] += np.asarray(res.results[c]["outT0"], np.float32).T
        out[b] += np.asarray(res.results[c]["outT1"], np.float32).T
    out += np.asarray(bo, np.float32)[None, None, :]
    return out
